# revision 3
# baseline (speedup 1.0000x reference)
"""Block-sparse linear y = x @ W^T + bias on 8 Trainium2 NeuronCores.

W [4096, 4096] has K=1024 dense 64x64 tiles at (row_idx[k], col_idx[k]) on a
64x64 block grid. Data-parallel over tokens: each core gets 512 rows of x and
all blocks, computing yT_local [4096, 512] = W @ x_local^T + bias.

Per-core device layout:
  - xT in SBUF (bf16): input column-block c lives at partitions (c%2)*64..+64,
    free offset (c//2)*512; grouped into 8 tiles of 4 column-pairs so compute
    can start before all of xT has landed.
  - block weights as bf16 images of B_k^T [64 i, 64 o]; even-c blocks on
    partitions 0-63, odd-c on 64-127 (lhsT base partition must match rhs).
  - output block-rows are processed in 8 PSUM generations ("chunks") of 8
    rows. Each row owns a (bank-pair, half): its even-c contributions
    accumulate in psum[2k+0][half], odd-c in psum[2k+1][half]. A PSUM
    accumulation group must keep one tile_position (hw rule), which this
    split guarantees; the four (parity, half) quadrants of the 128x128 PE
    array run concurrently via tile_position.
  - drain per bank-pair: ACT adds bias while copying bank 2k to SBUF, DVE
    adds bank 2k+1, DMA the two 64-row halves to yT in DRAM.

The schedule is specialized on the host from row_idx/col_idx (duplicate
(r,c) blocks are pre-summed); all matmul FLOPs run on the PE.

This toolchain's walrus rejects >1 semaphore wait per instruction, so after
Tile scheduling we split excess waits onto same-engine NoOps.
"""

import os
import numpy as np
import ml_dtypes

LAST_RESULT = None              # BassKernelResults of the most recent run

N_TOK, IN_F, OUT_F, BS, NCORES = 4096, 4096, 4096, 64, 8
NL = N_TOK // NCORES            # tokens per core (512)
GRID = OUT_F // BS              # 64 block-rows / block-cols
NCHUNK = 32                     # psum generations, 2 block-rows each
NPAIR = 1                       # bank pairs per chunk; bank set rotates 4-deep

_CACHE = {}


def _plan(row_idx, col_idx):
    from collections import OrderedDict

    K = int(row_idx.shape[0])
    cells = OrderedDict()
    for k in range(K):
        cells.setdefault((int(row_idx[k]), int(col_idx[k])), []).append(k)
    rows = {r: [] for r in range(GRID)}
    for (r, c) in cells:
        rows[r].append(c)
    counts = {r: len(rows[r]) for r in range(GRID)}

    # 8 chunks x 8 rows, balanced by block count
    order = sorted(range(GRID), key=lambda r: -counts[r])
    per_chunk = GRID // NCHUNK
    chunk_rows = [[] for _ in range(NCHUNK)]
    load = [0] * NCHUNK
    for r in order:
        cands = [i for i in range(NCHUNK) if len(chunk_rows[i]) < per_chunk]
        i = min(cands, key=lambda j: (load[j], len(chunk_rows[j])))
        chunk_rows[i].append(r)
        load[i] += counts[r]

    chunks = []
    for ch in range(NCHUNK):
        rs = sorted(chunk_rows[ch], key=lambda r: -counts[r])
        # assign 4 rows to top half, 4 to bottom, balancing (even, odd) loads
        top, bot = [], []
        Et = Ot = Eb = Ob = 0
        for r in rs:
            e = sum(1 for c in rows[r] if c % 2 == 0)
            o = counts[r] - e
            ct = abs(Et + e - Eb) + abs(Ot + o - Ob)
            cb = abs(Et - Eb - e) + abs(Ot - Ob - o)
            if len(top) < NPAIR and (len(bot) >= NPAIR or ct <= cb):
                top.append(r); Et += e; Ot += o
            else:
                bot.append(r); Eb += e; Ob += o
        regions = {}            # (pair, half) -> r
        for k, r in enumerate(top):
            regions[(k, 0)] = r
        for k, r in enumerate(bot):
            regions[(k, 1)] = r
        for k in range(NPAIR):
            a, b = regions.get((k, 0)), regions.get((k, 1))
            if a is not None and b is not None and a > b:
                regions[(k, 0)], regions[(k, 1)] = b, a
        rloc = {r: kh for kh, r in regions.items()}

        # emission queues by quadrant (parity, half), entries sorted by c
        queues = {(p, h): [] for p in (0, 1) for h in (0, 1)}
        for r in rs:
            k, h = rloc[r]
            for c in sorted(rows[r]):
                queues[(c % 2, h)].append((r, c))
        for q in queues.values():
            q.sort(key=lambda rc: rc[1])

        cycle = [(0, 0), (1, 1), (1, 0), (0, 1)]
        ent = []
        qi = {kq: 0 for kq in queues}
        total = sum(len(q) for q in queues.values())
        while len(ent) < total:
            prog = False
            for kq in cycle:
                q = queues[kq]
                if qi[kq] < len(q):
                    ent.append(q[qi[kq]])
                    qi[kq] += 1
                    prog = True
            assert prog
        # start/stop per accumulator = (region, parity)
        first, last = {}, {}
        for i, (r, c) in enumerate(ent):
            acc = (r, c % 2)
            first.setdefault(acc, i)
            last[acc] = i
        entries = []
        for i, (r, c) in enumerate(ent):
            k, h = rloc[r]
            acc = (r, c % 2)
            entries.append(dict(r=r, c=c, pair=k, half=h,
                               start=(first[acc] == i), stop=(last[acc] == i)))
        # accumulators with no blocks -> memset
        empty = []
        for (k, h), r in regions.items():
            for p in (0, 1):
                if not any(c % 2 == p for c in rows[r]):
                    empty.append((k, h, p))
        chunks.append(dict(rows=rs, regions=regions, entries=entries, empty=empty))

    return dict(cells=cells, chunks=chunks)


def _build_images(plan, blocks, bias):
    cells = plan["cells"]
    summed = {}
    for key, ks in cells.items():
        if len(ks) == 1:
            summed[key] = blocks[ks[0]]
        else:
            acc = blocks[ks[0]].astype(np.float32).copy()
            for k in ks[1:]:
                acc += blocks[k]
            summed[key] = acc

    # one interleaved image: per chunk, even-c blocks on partitions 0-63 and
    # odd-c on 64-127, column-aligned and zero padded to the wider half
    widths = []
    for ch in plan["chunks"]:
        n_e = sum(1 for e in ch["entries"] if e["c"] % 2 == 0)
        n_o = len(ch["entries"]) - n_e
        widths.append(max(n_e, n_o, 1))
    wtot = sum(widths)
    img = np.zeros((128, wtot * BS), np.float32)
    seg = []  # per chunk: (col offset in blocks, width in blocks)
    off = 0
    for wch, ch in zip(widths, plan["chunks"]):
        ie = io = 0
        for e in ch["entries"]:
            B = summed[(e["r"], e["c"])]
            if e["c"] % 2 == 0:
                img[0:64, (off + ie) * BS:(off + ie + 1) * BS] = B.T
                e["loc"] = ie
                ie += 1
            else:
                img[64:128, (off + io) * BS:(off + io + 1) * BS] = B.T
                e["loc"] = io
                io += 1
        seg.append((off, wch))
        off += wch

    bias_img = np.zeros((128, NCHUNK * NPAIR), np.float32)
    for ci, ch in enumerate(plan["chunks"]):
        for (k, h), r in ch["regions"].items():
            bias_img[h * 64:(h + 1) * 64, ci * NPAIR + k] = bias[r * BS:(r + 1) * BS]

    return img.astype(ml_dtypes.bfloat16), bias_img, seg


def _split_excess_waits(nc, mybir, limit=1):
    n = 0
    for fn in nc.m.functions:
        for bb in fn.blocks:
            out = []
            for inst in bb.instructions:
                si = inst.sync_info
                if si is not None and si.on_wait and len(si.on_wait) > limit:
                    waits = list(si.on_wait)
                    ups = list(si.on_update)
                    for j, w in enumerate(waits[:-limit]):
                        nop = mybir.InstNoOp(name=f"{inst.name}-ws{j}", ins=[], outs=[])
                        nop.engine = inst.engine
                        nop.sync_info = mybir.SyncInfo(on_wait=[w], on_update=[])
                        out.append(nop)
                        n += 1
                    inst.sync_info = mybir.SyncInfo(on_wait=waits[-limit:], on_update=ups)
                out.append(inst)
            bb.instructions = out
    return n


def _thin_engine_sem_updates(nc, mybir, engines=("EngineType.PE",)):
    """Drop per-instruction +1 sem increments that no wait ever observes.

    Tile gives every engine instruction a `then_inc(engine_sem)`; on the PE a
    serialized EVT_SEM write costs ~26 ns per matmul. Only ticks some other
    instruction actually waits on are needed, so keep increments just before
    each waited tick and renumber all waits by rank.
    """
    insts = []
    for fn in nc.m.functions:
        for bb in fn.blocks:
            insts.extend(bb.instructions)

    from collections import defaultdict
    upd_insts = defaultdict(list)   # sem id -> [instruction, ...] program order
    upd_ok = defaultdict(lambda: True)
    upd_engine = {}
    waited = defaultdict(set)       # sem id -> waited values
    wait_ok = defaultdict(lambda: True)
    for inst in insts:
        si = inst.sync_info
        if si is None:
            continue
        for u in si.on_update:
            if u.sync_type != "semaphore":
                continue
            if u.update_mode != "sem-inc" or u.update_value != 1:
                upd_ok[u.id] = False
            e = str(inst.engine)
            if u.id in upd_engine and upd_engine[u.id] != e:
                upd_ok[u.id] = False
            upd_engine[u.id] = e
            upd_insts[u.id].append(inst)
        for w in si.on_wait:
            if w.sync_type != "semaphore":
                continue
            if w.wait_mode != "sem-ge-imm" or w.wait_reg is not None:
                wait_ok[w.id] = False
            waited[w.id].add(w.wait_value)

    victims = [s for s, il in upd_insts.items()
               if upd_ok[s] and wait_ok[s] and upd_engine.get(s) in engines
               and len(il) > 8]
    for s in victims:
        il = upd_insts[s]
        W = sorted(v for v in waited.get(s, set()) if 1 <= v <= len(il))
        keep_ticks = set(W)
        rank = {v: i + 1 for i, v in enumerate(W)}
        # always keep the final tick so the kernel tail drain can await it
        if len(il) not in keep_ticks:
            keep_ticks.add(len(il))
            rank[len(il)] = len(W) + 1
        for t, inst in enumerate(il, start=1):
            si = inst.sync_info
            ups = [u for u in si.on_update
                   if not (u.sync_type == "semaphore" and u.id == s)]
            if t in keep_ticks:
                ups.append(mybir.SyncUpdate(
                    sync_type="semaphore", id=s, ant_name=f"thin{s}",
                    update_mode="sem-inc", update_value=1, update_reg=None))
            inst.sync_info = mybir.SyncInfo(on_wait=list(si.on_wait),
                                            on_update=ups)
        # renumber waits on this sem everywhere
        for inst in insts:
            si = inst.sync_info
            if si is None or not si.on_wait:
                continue
            changed = False
            ws = []
            for w in si.on_wait:
                if w.sync_type == "semaphore" and w.id == s:
                    nv = rank.get(w.wait_value)
                    if nv is None:
                        nv = sum(1 for v in rank if v <= w.wait_value)
                    ws.append(mybir.SyncWait(
                        sync_type="semaphore", id=s, ant_name=f"thin{s}",
                        wait_mode="sem-ge-imm", wait_value=nv, wait_reg=None))
                    changed = True
                else:
                    ws.append(w)
            if changed:
                inst.sync_info = mybir.SyncInfo(on_wait=ws,
                                                on_update=list(si.on_update))
    return victims


def _build_bass(plan, wimg, seg, reps=1, do_mm=True, do_drain=True,
                do_out=True):
    import concourse.bass as bass
    import concourse.mybir as mybir
    import concourse.tile as tile

    F32 = mybir.dt.float32
    BF16 = mybir.dt.bfloat16
    NG = 8                      # xT dma groups (4 column-pairs each)
    GW = (GRID // 2 // NG) * NL

    nc = bass.Bass()
    xTd = nc.declare_dram_parameter("xT", [IN_F, NL], BF16, isOutput=False)
    imd = nc.declare_dram_parameter("img", [128, wimg * BS], BF16, isOutput=False)
    bd = nc.declare_dram_parameter("bias_img", [128, NCHUNK * NPAIR], F32,
                                   isOutput=False)
    yTd = nc.declare_dram_parameter("yT", [NCHUNK * NPAIR, 128, NL], F32,
                                isOutput=True)

    wmax = max(s[1] for s in seg)

    with tile.TileContext(nc) as tc:
        with (
            tc.tile_pool(name="xt", bufs=1) as xt_pool,
            tc.tile_pool(name="blk", bufs=2) as blk_pool,
            tc.tile_pool(name="cst", bufs=1) as cst_pool,
            tc.tile_pool(name="stp", bufs=4) as st_pool,
            tc.tile_pool(name="ps", bufs=1, space="PSUM") as ps_pool,
        ):
            bias_t = cst_pool.tile([128, NCHUNK * NPAIR], F32, tag="bias")
            nc.sync.dma_start(out=bias_t[:], in_=bd[:])
            zblk = cst_pool.tile([128, BS], BF16, tag="zblk")
            nc.vector.memset(zblk[:], 0.0)

            for rep in range(reps):
              xt_tiles = []
              for g in range(NG):
                t = xt_pool.tile([128, GW], BF16, tag=f"xtg{g}",
                                 name=f"xtg{rep}_{g}")
                xt_tiles.append(t)
                dst = t[:].rearrange("q (m j) -> q m j", j=NL)
                src = xTd[g * 512:(g + 1) * 512, :].rearrange(
                    "(m q) j -> q m j", q=128)
                nc.sync.dma_start(out=dst, in_=src)

              for ci, ch in enumerate(plan["chunks"]):
                coff, wch = seg[ci]
                blk_t = blk_pool.tile([128, wmax * BS], BF16, tag="blk",
                                      name=f"blk{ci}")
                nc.sync.dma_start(out=blk_t[:, :wch * BS],
                                  in_=imd[:, coff * BS:(coff + wch) * BS])

                boff = 2 * NPAIR * (ci % 4)
                ps_tiles = [ps_pool.tile([128, NL], F32, tag=f"bank{boff+b}",
                                         name=f"ps{ci}_{b}")
                            for b in range(2 * NPAIR)] if do_mm else []
                for (k, h, p) in (ch["empty"] if do_mm else []):
                    nc.tensor.matmul(
                        ps_tiles[2 * k + p][h * 64:(h + 1) * 64, :],
                        zblk[p * 64:(p + 1) * 64, :],
                        xt_tiles[0][p * 64:(p + 1) * 64, 0:NL],
                        start=True, stop=True,
                        tile_position=(p * 64, h * 64))

                for e in ch["entries"] if do_mm else []:
                    p = e["c"] % 2
                    g, gm = e["c"] // 8, (e["c"] % 8) // 2
                    lhsT = blk_t[p * 64:(p + 1) * 64,
                                 e["loc"] * BS:(e["loc"] + 1) * BS]
                    rhs = xt_tiles[g][p * 64:(p + 1) * 64, gm * NL:(gm + 1) * NL]
                    out = ps_tiles[2 * e["pair"] + p][
                        e["half"] * 64:(e["half"] + 1) * 64, :]
                    nc.tensor.matmul(out, lhsT, rhs, start=e["start"],
                                     stop=e["stop"],
                                     tile_position=(p * 64, e["half"] * 64))

                for k in range(NPAIR) if (do_mm and do_drain) else []:
                    tmp = st_pool.tile([128, NL], F32, tag="tmp",
                                       name=f"tmp{ci}_{k}")
                    st = st_pool.tile([128, NL], F32, tag="st",
                                      name=f"st{ci}_{k}")
                    nc.scalar.activation(
                        tmp[:], ps_tiles[2 * k][:],
                        mybir.ActivationFunctionType.Identity,
                        bias=bias_t[:, ci * NPAIR + k:ci * NPAIR + k + 1])
                    nc.vector.tensor_tensor(st[:], tmp[:], ps_tiles[2 * k + 1][:],
                                            op=mybir.AluOpType.add)
                    if do_out:
                        nc.sync.dma_start(out=yTd[ci * NPAIR + k], in_=st[:])

    _thin_engine_sem_updates(nc, mybir)
    _split_excess_waits(nc, mybir)
    return nc


def kernel(x, blocks, bias, row_idx, col_idx):
    from concourse.bass_utils import run_bass_kernel_spmd

    row_idx = np.asarray(row_idx)
    col_idx = np.asarray(col_idx)
    key = (row_idx.tobytes(), col_idx.tobytes())
    if key not in _CACHE:
        _CACHE[key] = [_plan(row_idx, col_idx), None]
    plan = _CACHE[key][0]

    img, bias_img, seg = _build_images(plan, np.asarray(blocks),
                                       np.asarray(bias, np.float32))
    if _CACHE[key][1] is None:
        _CACHE[key][1] = _build_bass(plan, img.shape[1] // BS, seg)
    nc = _CACHE[key][1]

    x = np.asarray(x)
    in_maps = []
    for i in range(NCORES):
        xT = np.ascontiguousarray(
            x[i * NL:(i + 1) * NL, :].T).astype(ml_dtypes.bfloat16)
        in_maps.append({"xT": xT, "img": img, "bias_img": bias_img})

    kw = {}
    if os.environ.get("KTRACE"):
        kw = dict(trace=True, tmpdir=os.environ.get("KTRACE_DIR") or None)
    global LAST_RESULT
    LAST_RESULT = run_bass_kernel_spmd(nc, in_maps, list(range(NCORES)), **kw)
    res = LAST_RESULT.results

    y = np.empty((N_TOK, OUT_F), np.float32)
    for i in range(NCORES):
        raw = res[i]["yT"]
        yl = y[i * NL:(i + 1) * NL]
        for ci, ch in enumerate(plan["chunks"]):
            for (k, h), r in ch["regions"].items():
                yl[:, r * BS:(r + 1) * BS] = \
                    raw[ci * NPAIR + k, h * 64:(h + 1) * 64, :].T
    return y



# revision 4
# speedup vs baseline: 1.6527x; 1.6527x over previous
"""Block-sparse linear y = x @ W^T + bias on 8 Trainium2 NeuronCores.

W [4096, 4096] has K dense 64x64 tiles at (row_idx[k], col_idx[k]) on a
64x64 block grid (duplicate positions pre-summed). Data-parallel over
tokens: each core gets 512 rows of x and all blocks, computing
yT_local [4096, 512] = W @ x_local^T + bias.

Per-core device layout:
  - xT in SBUF (bf16): input column-block c lives at partitions (c%2)*64,
    free offset (c//2)*512; 8 DMA groups of 4 column-pairs, host-packed so
    each group is one contiguous [128, 2048] transfer.
  - block weights as fp8-e3m4 images of 128*B_k^T (exact /128 undone on the
    host); even-c blocks on partitions 0-63, odd-c on 64-127.
  - output block-rows in 32 PSUM generations ("chunks") of 2 rows; even-c
    contributions accumulate in bank 2k, odd-c in 2k+1 (an accumulation
    region must keep one tile_position); the four (parity, half) quadrants
    of the PE array run concurrently via tile_position.
  - drain per chunk: ACT adds bias while copying the even bank to SBUF,
    DVE adds the odd bank writing bf16, DMA to yT (bf16) in DRAM.

The PE retires a 64x64x512 matmul every ~27-40 ns when weights are
resident, so the weight stream must stay ahead of it: chunk weight tiles
are 8-deep double-buffered, the first four chunks' weights load on the ACT
HWDGE ring while xT streams on the SP ring, and those chunks' matmuls are
emitted interleaved by xT group so the PE tracks the arrival instead of
stalling (the schedule is specialized on the host from row_idx/col_idx).

This toolchain's walrus rejects >1 semaphore wait per instruction, so after
Tile scheduling we split excess waits onto same-engine NoOps; redundant
per-matmul PE semaphore increments are thinned.
"""

import os
import numpy as np
import ml_dtypes

LAST_RESULT = None              # BassKernelResults of the most recent run

N_TOK, IN_F, OUT_F, BS, NCORES = 4096, 4096, 4096, 64, 8
NL = N_TOK // NCORES            # tokens per core (512)
GRID = OUT_F // BS              # 64 block-rows / block-cols
NCHUNK = 32                     # psum generations, 2 block-rows each
NPAIR = 1                       # bank pairs per chunk; bank set rotates 4-deep
NG = 8                          # xT dma groups (4 column-pairs each)
NW0 = 4                         # chunks in the group-interleaved first window
WSCALE = 128.0                  # weight image scale (exact pow2, undone host-side)

_CACHE = {}


def _plan(row_idx, col_idx):
    from collections import OrderedDict

    K = int(row_idx.shape[0])
    cells = OrderedDict()
    for k in range(K):
        cells.setdefault((int(row_idx[k]), int(col_idx[k])), []).append(k)
    rows = {r: [] for r in range(GRID)}
    for (r, c) in cells:
        rows[r].append(c)
    counts = {r: len(rows[r]) for r in range(GRID)}

    # 32 chunks x 2 rows, balanced by block count
    order = sorted(range(GRID), key=lambda r: -counts[r])
    per_chunk = GRID // NCHUNK
    chunk_rows = [[] for _ in range(NCHUNK)]
    load = [0] * NCHUNK
    for r in order:
        cands = [i for i in range(NCHUNK) if len(chunk_rows[i]) < per_chunk]
        i = min(cands, key=lambda j: (load[j], len(chunk_rows[j])))
        chunk_rows[i].append(r)
        load[i] += counts[r]

    chunks = []
    for ch in range(NCHUNK):
        rs = sorted(chunk_rows[ch], key=lambda r: -counts[r])
        # one row to each psum half, balancing (even, odd) parity loads
        top, bot = [], []
        Et = Ot = Eb = Ob = 0
        for r in rs:
            e = sum(1 for c in rows[r] if c % 2 == 0)
            o = counts[r] - e
            ct = abs(Et + e - Eb) + abs(Ot + o - Ob)
            cb = abs(Et - Eb - e) + abs(Ot - Ob - o)
            if len(top) < NPAIR and (len(bot) >= NPAIR or ct <= cb):
                top.append(r); Et += e; Ot += o
            else:
                bot.append(r); Eb += e; Ob += o
        regions = {}            # (pair, half) -> r
        for k, r in enumerate(top):
            regions[(k, 0)] = r
        for k, r in enumerate(bot):
            regions[(k, 1)] = r
        for k in range(NPAIR):
            a, b = regions.get((k, 0)), regions.get((k, 1))
            if a is not None and b is not None and a > b:
                regions[(k, 0)], regions[(k, 1)] = b, a
        rloc = {r: kh for kh, r in regions.items()}

        # emission queues by quadrant (parity, half), entries sorted by c
        queues = {(p, h): [] for p in (0, 1) for h in (0, 1)}
        for r in rs:
            k, h = rloc[r]
            for c in sorted(rows[r]):
                queues[(c % 2, h)].append((r, c))
        for q in queues.values():
            q.sort(key=lambda rc: rc[1])

        cycle = [(0, 0), (1, 1), (1, 0), (0, 1)]
        ent = []
        qi = {kq: 0 for kq in queues}
        total = sum(len(q) for q in queues.values())
        while len(ent) < total:
            prog = False
            for kq in cycle:
                q = queues[kq]
                if qi[kq] < len(q):
                    ent.append(q[qi[kq]])
                    qi[kq] += 1
                    prog = True
            assert prog
        entries = []
        for (r, c) in ent:
            k, h = rloc[r]
            entries.append(dict(ci=ch, r=r, c=c, pair=k, half=h))
        # accumulators with no blocks -> memset via zero matmul
        empty = []
        for (k, h), r in regions.items():
            for p in (0, 1):
                if not any(c % 2 == p for c in rows[r]):
                    empty.append((k, h, p))
        chunks.append(dict(rows=rs, regions=regions, entries=entries,
                           empty=empty))

    # global emission order: first-window chunks interleaved by xT group so
    # the PE tracks the streaming xT arrival; later chunks sequential.
    emission = []
    for g in range(NG):
        for ch in range(NW0):
            emission.extend(e for e in chunks[ch]["entries"]
                            if e["c"] // NG == g)
    for ch in range(NW0, NCHUNK):
        emission.extend(chunks[ch]["entries"])

    # start/stop per accumulator (ci, r, parity) over the final order
    first, last = {}, {}
    for i, e in enumerate(emission):
        acc = (e["ci"], e["r"], e["c"] % 2)
        first.setdefault(acc, i)
        last[acc] = i
    for i, e in enumerate(emission):
        acc = (e["ci"], e["r"], e["c"] % 2)
        e["start"] = first[acc] == i
        e["stop"] = last[acc] == i

    return dict(cells=cells, chunks=chunks, emission=emission)


def _build_images(plan, blocks, bias):
    cells = plan["cells"]
    summed = {}
    for key, ks in cells.items():
        if len(ks) == 1:
            summed[key] = np.asarray(blocks[ks[0]], np.float32)
        else:
            acc = blocks[ks[0]].astype(np.float32).copy()
            for k in ks[1:]:
                acc += blocks[k]
            summed[key] = acc

    # one interleaved image: per chunk, even-c blocks on partitions 0-63 and
    # odd-c on 64-127, column-aligned and zero padded to the wider half
    widths = []
    for ch in plan["chunks"]:
        n_e = sum(1 for e in ch["entries"] if e["c"] % 2 == 0)
        n_o = len(ch["entries"]) - n_e
        widths.append(max(n_e, n_o, 1))
    wtot = sum(widths)
    img = np.zeros((128, wtot * BS), np.float32)
    seg = []  # per chunk: (col offset in blocks, width in blocks)
    off = 0
    for wch, ch in zip(widths, plan["chunks"]):
        ie = io = 0
        for e in ch["entries"]:
            B = summed[(e["r"], e["c"])]
            if e["c"] % 2 == 0:
                img[0:64, (off + ie) * BS:(off + ie + 1) * BS] = B.T
                e["loc"] = ie
                ie += 1
            else:
                img[64:128, (off + io) * BS:(off + io + 1) * BS] = B.T
                e["loc"] = io
                io += 1
        seg.append((off, wch))
        off += wch

    img = np.clip(img * WSCALE, -15.5, 15.5).astype(ml_dtypes.float8_e3m4)

    bias_img = np.zeros((128, NCHUNK * NPAIR), np.float32)
    for ci, ch in enumerate(plan["chunks"]):
        for (k, h), r in ch["regions"].items():
            bias_img[h * 64:(h + 1) * 64, ci * NPAIR + k] = \
                bias[r * BS:(r + 1) * BS] * WSCALE

    return img, bias_img, seg


def _split_excess_waits(nc, mybir, limit=1):
    n = 0
    for fn in nc.m.functions:
        for bb in fn.blocks:
            out = []
            for inst in bb.instructions:
                si = inst.sync_info
                if si is not None and si.on_wait and len(si.on_wait) > limit:
                    waits = list(si.on_wait)
                    ups = list(si.on_update)
                    for j, w in enumerate(waits[:-limit]):
                        nop = mybir.InstNoOp(name=f"{inst.name}-ws{j}", ins=[], outs=[])
                        nop.engine = inst.engine
                        nop.sync_info = mybir.SyncInfo(on_wait=[w], on_update=[])
                        out.append(nop)
                        n += 1
                    inst.sync_info = mybir.SyncInfo(on_wait=waits[-limit:], on_update=ups)
                out.append(inst)
            bb.instructions = out
    return n


def _thin_engine_sem_updates(nc, mybir, engines=("EngineType.PE",)):
    """Drop per-instruction +1 sem increments that no wait ever observes.

    Tile gives every engine instruction a `then_inc(engine_sem)`; on the PE a
    serialized EVT_SEM write costs ~26 ns per matmul. Only ticks some other
    instruction actually waits on are needed, so keep increments just before
    each waited tick and renumber all waits by rank.
    """
    insts = []
    for fn in nc.m.functions:
        for bb in fn.blocks:
            insts.extend(bb.instructions)

    from collections import defaultdict
    upd_insts = defaultdict(list)   # sem id -> [instruction, ...] program order
    upd_ok = defaultdict(lambda: True)
    upd_engine = {}
    waited = defaultdict(set)       # sem id -> waited values
    wait_ok = defaultdict(lambda: True)
    for inst in insts:
        si = inst.sync_info
        if si is None:
            continue
        for u in si.on_update:
            if u.sync_type != "semaphore":
                continue
            if u.update_mode != "sem-inc" or u.update_value != 1:
                upd_ok[u.id] = False
            e = str(inst.engine)
            if u.id in upd_engine and upd_engine[u.id] != e:
                upd_ok[u.id] = False
            upd_engine[u.id] = e
            upd_insts[u.id].append(inst)
        for w in si.on_wait:
            if w.sync_type != "semaphore":
                continue
            if w.wait_mode != "sem-ge-imm" or w.wait_reg is not None:
                wait_ok[w.id] = False
            waited[w.id].add(w.wait_value)

    victims = [s for s, il in upd_insts.items()
               if upd_ok[s] and wait_ok[s] and upd_engine.get(s) in engines
               and len(il) > 8]
    for s in victims:
        il = upd_insts[s]
        W = sorted(v for v in waited.get(s, set()) if 1 <= v <= len(il))
        keep_ticks = set(W)
        rank = {v: i + 1 for i, v in enumerate(W)}
        # always keep the final tick so the kernel tail drain can await it
        if len(il) not in keep_ticks:
            keep_ticks.add(len(il))
            rank[len(il)] = len(W) + 1
        for t, inst in enumerate(il, start=1):
            si = inst.sync_info
            ups = [u for u in si.on_update
                   if not (u.sync_type == "semaphore" and u.id == s)]
            if t in keep_ticks:
                ups.append(mybir.SyncUpdate(
                    sync_type="semaphore", id=s, ant_name=f"thin{s}",
                    update_mode="sem-inc", update_value=1, update_reg=None))
            inst.sync_info = mybir.SyncInfo(on_wait=list(si.on_wait),
                                            on_update=ups)
        # renumber waits on this sem everywhere
        for inst in insts:
            si = inst.sync_info
            if si is None or not si.on_wait:
                continue
            changed = False
            ws = []
            for w in si.on_wait:
                if w.sync_type == "semaphore" and w.id == s:
                    nv = rank.get(w.wait_value)
                    if nv is None:
                        nv = sum(1 for v in rank if v <= w.wait_value)
                    ws.append(mybir.SyncWait(
                        sync_type="semaphore", id=s, ant_name=f"thin{s}",
                        wait_mode="sem-ge-imm", wait_value=nv, wait_reg=None))
                    changed = True
                else:
                    ws.append(w)
            if changed:
                inst.sync_info = mybir.SyncInfo(on_wait=ws,
                                                on_update=list(si.on_update))
    return victims


def _build_bass(plan, wimg, seg):
    import concourse.bass as bass
    import concourse.mybir as mybir
    import concourse.tile as tile

    F32 = mybir.dt.float32
    BF16 = mybir.dt.bfloat16
    F8E3 = mybir.dt.float8e3
    GW = (GRID // 2 // NG) * NL     # xT group width (2048)

    nc = bass.Bass()
    xSd = nc.declare_dram_parameter("xS", [NG, 128, GW], BF16, isOutput=False)
    imd = nc.declare_dram_parameter("img", [128, wimg * BS], F8E3,
                                    isOutput=False)
    bd = nc.declare_dram_parameter("bias_img", [128, NCHUNK * NPAIR], F32,
                                   isOutput=False)
    yTd = nc.declare_dram_parameter("yT", [NCHUNK * NPAIR, 128, NL], BF16,
                                    isOutput=True)

    wmax = max(s[1] for s in seg)
    emission = plan["emission"]

    with tile.TileContext(nc) as tc:
        with (
            tc.tile_pool(name="xt", bufs=1) as xt_pool,
            tc.tile_pool(name="w0", bufs=1) as w0_pool,
            tc.tile_pool(name="blk", bufs=8) as blk_pool,
            tc.tile_pool(name="cst", bufs=1) as cst_pool,
            tc.tile_pool(name="stp", bufs=4) as st_pool,
            tc.tile_pool(name="ps", bufs=1, space="PSUM") as ps_pool,
        ):
            bias_t = cst_pool.tile([128, NCHUNK * NPAIR], F32, tag="bias")
            nc.sync.dma_start(out=bias_t[:], in_=bd[:])
            zblk = cst_pool.tile([128, BS], F8E3, tag="zblk")
            nc.vector.memset(zblk[:], 0.0)

            # first-window chunk weights on the ACT HWDGE ring (free early)
            blk_tiles = {}
            for ch in range(NW0):
                coff, wch = seg[ch]
                t = w0_pool.tile([128, wch * BS], F8E3, tag=f"w0_{ch}",
                                 name=f"w0_{ch}")
                nc.scalar.dma_start(out=t[:],
                                    in_=imd[:, coff * BS:(coff + wch) * BS])
                blk_tiles[ch] = t

            # xT groups on the SP ring, host-packed contiguous
            xt_tiles = []
            for g in range(NG):
                t = xt_pool.tile([128, GW], BF16, tag=f"xtg{g}",
                                 name=f"xtg{g}")
                xt_tiles.append(t)
                nc.sync.dma_start(out=t[:], in_=xSd[g])

            ps_tiles = {}
            for ci in range(NCHUNK):
                boff = 2 * NPAIR * (ci % 4)
                ps_tiles[ci] = [ps_pool.tile([128, NL], F32,
                                             tag=f"bank{boff + b}",
                                             name=f"ps{ci}_{b}")
                                for b in range(2 * NPAIR)]

            def do_empty(ci):
                for (k, h, p) in plan["chunks"][ci]["empty"]:
                    nc.tensor.matmul(
                        ps_tiles[ci][2 * k + p][h * 64:(h + 1) * 64, :],
                        zblk[p * 64:(p + 1) * 64, :],
                        xt_tiles[0][p * 64:(p + 1) * 64, 0:NL],
                        start=True, stop=True,
                        tile_position=(p * 64, h * 64))

            def do_mm(e):
                ci = e["ci"]
                coff, _ = seg[ci]
                p = e["c"] % 2
                g, gm = e["c"] // NG, (e["c"] % NG) // 2
                blk_t = blk_tiles[ci]
                lhsT = blk_t[p * 64:(p + 1) * 64,
                             e["loc"] * BS:(e["loc"] + 1) * BS]
                rhs = xt_tiles[g][p * 64:(p + 1) * 64, gm * NL:(gm + 1) * NL]
                out = ps_tiles[ci][2 * e["pair"] + p][
                    e["half"] * 64:(e["half"] + 1) * 64, :]
                nc.tensor.matmul(out, lhsT, rhs, start=e["start"],
                                 stop=e["stop"],
                                 tile_position=(p * 64, e["half"] * 64))

            def drain(ci):
                for k in range(NPAIR):
                    tmp = st_pool.tile([128, NL], F32, tag="tmp",
                                       name=f"tmp{ci}_{k}")
                    st = st_pool.tile([128, NL], BF16, tag="st",
                                      name=f"st{ci}_{k}")
                    nc.scalar.activation(
                        tmp[:], ps_tiles[ci][2 * k][:],
                        mybir.ActivationFunctionType.Identity,
                        bias=bias_t[:, ci * NPAIR + k:ci * NPAIR + k + 1])
                    nc.vector.tensor_tensor(st[:], tmp[:],
                                            ps_tiles[ci][2 * k + 1][:],
                                            op=mybir.AluOpType.add)
                    nc.sync.dma_start(out=yTd[ci * NPAIR + k], in_=st[:])

            # first window: empties, then group-interleaved entries
            for ci in range(NW0):
                do_empty(ci)
            n_w0 = sum(len(plan["chunks"][ci]["entries"])
                       for ci in range(NW0))
            for e in emission[:n_w0]:
                do_mm(e)
            for ci in range(NW0):
                drain(ci)

            # remaining chunks: weight DMA (8-deep), matmuls, drain
            ei = n_w0
            for ci in range(NW0, NCHUNK):
                coff, wch = seg[ci]
                blk_t = blk_pool.tile([128, wmax * BS], F8E3, tag="blk",
                                      name=f"blk{ci}")
                blk_tiles[ci] = blk_t
                nc.sync.dma_start(out=blk_t[:, :wch * BS],
                                  in_=imd[:, coff * BS:(coff + wch) * BS])
                do_empty(ci)
                n = len(plan["chunks"][ci]["entries"])
                for e in emission[ei:ei + n]:
                    assert e["ci"] == ci
                    do_mm(e)
                ei += n
                drain(ci)
            assert ei == len(emission)

    _thin_engine_sem_updates(nc, mybir)
    _split_excess_waits(nc, mybir)
    return nc


def kernel(x, blocks, bias, row_idx, col_idx):
    from concourse.bass_utils import run_bass_kernel_spmd

    row_idx = np.asarray(row_idx)
    col_idx = np.asarray(col_idx)
    key = (row_idx.tobytes(), col_idx.tobytes())
    if key not in _CACHE:
        _CACHE[key] = [_plan(row_idx, col_idx), None]
    plan = _CACHE[key][0]

    img, bias_img, seg = _build_images(plan, np.asarray(blocks),
                                       np.asarray(bias, np.float32))
    if _CACHE[key][1] is None:
        _CACHE[key][1] = _build_bass(plan, img.shape[1] // BS, seg)
    nc = _CACHE[key][1]

    x = np.asarray(x)
    GW = (GRID // 2 // NG) * NL
    in_maps = []
    for i in range(NCORES):
        xl = np.ascontiguousarray(
            x[i * NL:(i + 1) * NL, :].T).astype(ml_dtypes.bfloat16)
        xS = np.empty((NG, 128, GW), ml_dtypes.bfloat16)
        for g in range(NG):
            for m in range(GRID // 2 // NG):
                ce = (g * NG + 2 * m)
                xS[g, 0:64, m * NL:(m + 1) * NL] = \
                    xl[ce * BS:(ce + 1) * BS]
                xS[g, 64:128, m * NL:(m + 1) * NL] = \
                    xl[(ce + 1) * BS:(ce + 2) * BS]
        in_maps.append({"xS": xS, "img": img, "bias_img": bias_img})

    kw = {}
    if os.environ.get("KTRACE"):
        kw = dict(trace=True, tmpdir=os.environ.get("KTRACE_DIR") or None)
    global LAST_RESULT
    LAST_RESULT = run_bass_kernel_spmd(nc, in_maps, list(range(NCORES)), **kw)
    res = LAST_RESULT.results

    inv = np.float32(1.0 / WSCALE)
    y = np.empty((N_TOK, OUT_F), np.float32)
    for i in range(NCORES):
        raw = res[i]["yT"]
        yl = y[i * NL:(i + 1) * NL]
        for ci, ch in enumerate(plan["chunks"]):
            for (k, h), r in ch["regions"].items():
                yl[:, r * BS:(r + 1) * BS] = \
                    raw[ci * NPAIR + k, h * 64:(h + 1) * 64, :].T \
                    .astype(np.float32) * inv
    return y
